# revision 7
# baseline (speedup 1.0000x reference)
"""Trainium2 Bass kernel: DiGCN attention layer, B=8 L=2048 H=768.

Sharding: data-parallel over batch — one batch element per NeuronCore (8 cores).
Each core computes its whole [L,L] attention block locally; no collectives.

Device algorithm (per core, batch element b):
  u = h @ h.T / sqrt(H)  is symmetric, so exp(u) tiles computed directly in
  [m-partitions, l-free] ("transposed") layout — the layout the context bmm
  needs, since the TensorEngine contracts over the partition axis.
  LayerNorm is invariant to per-row positive scaling, so the softmax
  denominator S[l], the adjacency renorm sum T[l] and the +1e-10 term form a
  common per-row factor that cancels exactly — only unnormalized numerators
  N[m,l] = exp(u[m,l]) * A[l,m] are ever computed:
     ctx_raw[l,:] = sum_{m>l} N[m,l] hL[m,:] + N[l,l] hS[l,:] + sum_{m<l} N[m,l] hR[m,:]
  followed by LayerNorm(+gamma/beta) and ReLU.
  A^T tiles come from a f32->bf16 cast in DRAM + hardware DMA-transpose loads
  (the xbar transpose path is 2-byte only). All matmuls run in bf16 with f32
  PSUM accumulation; softmax/exp/LN math stays f32.
"""

import numpy as np
import ml_dtypes

B, L, H = 8, 2048, 768
P = 128
NJ = L // P          # 16 partition chunks of the contraction index m
NS = 4               # l supertiles
ST = L // NS         # 512 l-columns per supertile
NC_PER_ST = ST // P  # 4 l-chunks per supertile
SCALE = 1.0 / float(np.sqrt(H))
LN_EPS = 1e-12

_CACHE = {}


def _build(apply_gamma_beta: bool):
    import concourse.bacc as bacc
    import concourse.tile as tile
    from concourse import mybir
    from concourse.alu_op_type import AluOpType as alu

    f32 = mybir.dt.float32
    bf16 = mybir.dt.bfloat16
    AF = mybir.ActivationFunctionType

    nc = bacc.Bacc(trn_type="TRN2", target_bir_lowering=False, debug=False)

    h_in = nc.dram_tensor("h", [L, H], f32, kind="ExternalInput")
    a_in = nc.dram_tensor("A", [L, L], f32, kind="ExternalInput")
    w_in = {x: nc.dram_tensor(f"w{x}", [H, H], f32, kind="ExternalInput")
            for x in "lsr"}
    b_in = {x: nc.dram_tensor(f"b{x}", [1, H], f32, kind="ExternalInput")
            for x in "lsr"}
    lowmask_in = nc.dram_tensor("lowmask", [P, P], bf16, kind="ExternalInput")
    upmask_in = nc.dram_tensor("upmask", [P, P], bf16, kind="ExternalInput")
    idmask_in = nc.dram_tensor("idmask", [P, P], bf16, kind="ExternalInput")
    if apply_gamma_beta:
        g_in = nc.dram_tensor("gamma", [1, H], f32, kind="ExternalInput")
        beta_in = nc.dram_tensor("beta", [1, H], f32, kind="ExternalInput")
    out_dram = nc.dram_tensor("out", [L, H], f32, kind="ExternalOutput")

    import concourse.bass as bass

    def bcast_ap(src, n=P):
        # [1, F] DRAM tensor replicated across n partitions (partition step 0)
        ap = src[:]
        return bass.AP(tensor=ap.tensor, offset=ap.offset, ap=[[0, n]] + list(ap.ap[1:]))

    with tile.TileContext(nc) as tc:
        with (
            tc.tile_pool(name="persist", bufs=1) as persist,
            tc.tile_pool(name="dram", bufs=1, space="DRAM") as dram,
            tc.tile_pool(name="atp", bufs=1) as atp,
            tc.tile_pool(name="np_pool", bufs=1) as np_pool,
            tc.tile_pool(name="esb", bufs=1) as esb_pool,
            tc.tile_pool(name="small", bufs=1) as small,
            tc.tile_pool(name="epi", bufs=1) as epi,
            tc.tile_pool(name="psum_e", bufs=3, space="PSUM") as psum_e_pool,
            tc.tile_pool(name="psum_b", bufs=2, space="PSUM") as psum_b_pool,
        ):
            # ---- constants ----
            lowmask = persist.tile([P, P], bf16, tag="lowmask", name="lowmask_t")
            upmask = persist.tile([P, P], bf16, tag="upmask", name="upmask_t")
            idmask = persist.tile([P, P], bf16, tag="idmask", name="idmask_t")
            nc.sync.dma_start(out=lowmask[:], in_=lowmask_in[:])
            nc.sync.dma_start(out=upmask[:], in_=upmask_in[:])
            nc.sync.dma_start(out=idmask[:], in_=idmask_in[:])
            eps_t = persist.tile([P, 1], f32, tag="eps", name="eps_t")
            nc.vector.memset(eps_t[:], LN_EPS)
            # broadcasts first on the SWDGE FIFO — tiny, needed mid-kernel
            b_bc = {}
            for x in "lsr":
                b_bc[x] = persist.tile([P, H], f32, tag=f"bbc{x}", name=f"bbc{x}_t")
                nc.gpsimd.dma_start(out=b_bc[x][:], in_=bcast_ap(b_in[x]))
            if apply_gamma_beta:
                g_bc = persist.tile([P, H], f32, tag="gbc", name="gbc_t")
                beta_bc = persist.tile([P, H], f32, tag="betabc", name="betabc_t")
                nc.gpsimd.dma_start(out=g_bc[:], in_=bcast_ap(g_in))
                nc.gpsimd.dma_start(out=beta_bc[:], in_=bcast_ap(beta_in))

            # ---- DRAM bf16 scratch (SWDGE cast), ordered for pipelined startup:
            # wl first (first projection weight), then h column-strips (each
            # unblocks one hT transpose), then ws/wr, then A row-strips (strip s
            # unblocks all of supertile s's A^T tiles).
            w_bf = {}
            w_bf["l"] = dram.tile([H, H], bf16, tag="w_bfl", name="w_bfl")
            nc.gpsimd.dma_start(out=w_bf["l"][:], in_=w_in["l"][:])
            h_bf = dram.tile([L, H], bf16, tag="h_bf", name="h_bf")
            for d in range(H // P):
                nc.gpsimd.dma_start(out=h_bf[:, d * P:(d + 1) * P],
                                    in_=h_in[:, d * P:(d + 1) * P])
            for x in "sr":
                w_bf[x] = dram.tile([H, H], bf16, tag=f"w_bf{x}", name=f"w_bf{x}")
                nc.gpsimd.dma_start(out=w_bf[x][:], in_=w_in[x][:])
            a_bf = dram.tile([L, L], bf16, tag="a_bf", name="a_bf")
            for s in range(NS):
                nc.gpsimd.dma_start(out=a_bf[s * ST:(s + 1) * ST, :],
                                    in_=a_in[s * ST:(s + 1) * ST, :])
            hs_dram = dram.tile([L, H], bf16, tag="hs_dram", name="hs_dram")

            # ---- transposed loads (HWDGE xbar); sync queue is FIFO, so order
            # by when the cast dependency lands ----
            hT = [None] * (H // P)
            hT[0] = persist.tile([P, L], bf16, tag="hT0", name="hT0")
            nc.sync.dma_start(out=hT[0][:], in_=h_bf[:, 0:P], transpose=True)

            # ---- projections hL/hS/hR = h @ W^T + b (bf16 out) ----
            hX = {"l": [], "r": []}
            # W^T tiles live only through the projections; they share slots
            # (same tag) with the later N' numerator tiles.
            NP_BUFS = 34
            wT = {x: [] for x in "lsr"}
            for d in range(H // P):
                t = np_pool.tile([P, H], bf16, tag="np", bufs=NP_BUFS, name=f"wTl{d}")
                nc.sync.dma_start(out=t[:], in_=w_bf["l"][:, d * P:(d + 1) * P],
                                  transpose=True)
                wT["l"].append(t)
            for d in range(1, H // P):
                hT[d] = persist.tile([P, L], bf16, tag=f"hT{d}", name=f"hT{d}")
                nc.sync.dma_start(out=hT[d][:], in_=h_bf[:, d * P:(d + 1) * P],
                                  transpose=True)
            for x in "sr":
                for d in range(H // P):
                    t = np_pool.tile([P, H], bf16, tag="np", bufs=NP_BUFS,
                                     name=f"wT{x}{d}")
                    nc.sync.dma_start(out=t[:], in_=w_bf[x][:, d * P:(d + 1) * P],
                                      transpose=True)
                    wT[x].append(t)
            for x in "lsr":
                for m in range(NJ):
                    psum_p = psum_b_pool.tile([P, H], f32, tag="psb", name=f"pp{x}{m}")
                    for d in range(H // P):
                        lhsT = hT[d][:, m * P:(m + 1) * P]
                        nc.tensor.matmul(psum_p[:, 0:512], lhsT, wT[x][d][:, 0:512],
                                         start=(d == 0), stop=(d == H // P - 1))
                        nc.tensor.matmul(psum_p[:, 512:768], lhsT, wT[x][d][:, 512:768],
                                         start=(d == 0), stop=(d == H // P - 1))
                    if x == "s":
                        stage = small.tile([P, H], bf16, tag="hs_stage", bufs=3,
                                           name=f"hss{m}")
                        nc.vector.scalar_tensor_tensor(
                            out=stage[:], in0=psum_p[:], scalar=1.0, in1=b_bc[x][:],
                            op0=alu.mult, op1=alu.add)
                        nc.scalar.dma_start(out=hs_dram[m * P:(m + 1) * P, :],
                                            in_=stage[:])
                    else:
                        t = persist.tile([P, H], bf16, tag=f"h{x}{m}", name=f"h{x}{m}")
                        nc.vector.scalar_tensor_tensor(
                            out=t[:], in0=psum_p[:], scalar=1.0, in1=b_bc[x][:],
                            op0=alu.mult, op1=alu.add)
                        hX[x].append(t)

            # ---- attention numerator tiles + context bmm, pipelined over supertiles ----
            np_tiles = {}       # (s, j) -> [P, ST] bf16 numerator tiles (m-part, l-free)
            at_tiles = {}       # (s, j) -> [P, ST] bf16 A^T tiles
            diag_num = {}       # i -> [P, 1] f32: exp(u_ll) * A_ll for l-chunk i
            nl_diag = {}        # i -> [P, P] bf16 strictly-lower (m>l) masked diag block
            nr_diag = {}

            def e_phase(s):
                for j in range(NJ):
                    at_t = atp.tile([P, ST], bf16, tag="at", bufs=33, name=f"at{s}_{j}")
                    nc.sync.dma_start(
                        out=at_t[:], in_=a_bf[s * ST:(s + 1) * ST, j * P:(j + 1) * P],
                        transpose=True)
                    at_tiles[(s, j)] = at_t
                    psum_e = psum_e_pool.tile([P, ST], f32, tag="pse", bufs=3,
                                              name=f"pe{s}_{j}")
                    for d in range(H // P):
                        nc.tensor.matmul(psum_e[:], hT[d][:, j * P:(j + 1) * P],
                                         hT[d][:, s * ST:(s + 1) * ST],
                                         start=(d == 0), stop=(d == H // P - 1))
                    e_sb = esb_pool.tile([P, ST], bf16, tag="esb", bufs=4, name=f"e{s}_{j}")
                    nc.scalar.activation(out=e_sb[:], in_=psum_e[:], func=AF.Exp,
                                         scale=SCALE)
                    np_t = np_pool.tile([P, ST], bf16, tag="np", bufs=NP_BUFS,
                                        name=f"n{s}_{j}")
                    nc.vector.tensor_tensor(out=np_t[:], in0=e_sb[:], in1=at_t[:],
                                            op=alu.mult)
                    np_tiles[(s, j)] = np_t
                    if s * NC_PER_ST <= j < (s + 1) * NC_PER_ST:
                        c = j - s * NC_PER_ST
                        i = j
                        eblk = e_sb[:, c * P:(c + 1) * P]
                        ablk = at_t[:, c * P:(c + 1) * P]
                        junk = small.tile([P, P], bf16, tag="djunk", bufs=2,
                                          name=f"dj{i}")
                        ed = small.tile([P, 1], f32, tag="ediag", bufs=8, name=f"ed{i}")
                        nc.vector.scalar_tensor_tensor(
                            out=junk[:], in0=eblk, scalar=1.0, in1=idmask[:],
                            op0=alu.mult, op1=alu.mult, accum_out=ed[:])
                        junk2 = small.tile([P, P], bf16, tag="djunk", bufs=2,
                                           name=f"dj2{i}")
                        ad = small.tile([P, 1], f32, tag="adiag", bufs=8, name=f"ad{i}")
                        nc.vector.scalar_tensor_tensor(
                            out=junk2[:], in0=ablk, scalar=1.0, in1=idmask[:],
                            op0=alu.mult, op1=alu.mult, accum_out=ad[:])
                        dn = small.tile([P, 1], f32, tag="dnum", bufs=8, name=f"dn{i}")
                        nc.vector.tensor_tensor(out=dn[:], in0=ed[:], in1=ad[:],
                                                op=alu.mult)
                        diag_num[i] = dn
                        nl = small.tile([P, P], bf16, tag="nld", bufs=8, name=f"nl{i}")
                        nc.vector.tensor_tensor(out=nl[:], in0=np_t[:, c * P:(c + 1) * P],
                                                in1=lowmask[:], op=alu.mult)
                        nl_diag[i] = nl
                        nr = small.tile([P, P], bf16, tag="nrd", bufs=8, name=f"nr{i}")
                        nc.vector.tensor_tensor(out=nr[:], in0=np_t[:, c * P:(c + 1) * P],
                                                in1=upmask[:], op=alu.mult)
                        nr_diag[i] = nr

            def bmm_phase(s):
                for c in range(NC_PER_ST):
                    i = s * NC_PER_ST + c
                    psum_c = psum_b_pool.tile([P, H], f32, tag="psb", name=f"pc{i}")
                    pairs = []
                    for j in range(i + 1, NJ):
                        pairs.append((np_tiles[(s, j)][:, c * P:(c + 1) * P], hX["l"][j]))
                    pairs.append((nl_diag[i][:], hX["l"][i]))
                    pairs.append((nr_diag[i][:], hX["r"][i]))
                    for j in range(0, i):
                        pairs.append((np_tiles[(s, j)][:, c * P:(c + 1) * P], hX["r"][j]))
                    n = len(pairs)
                    for k, (lhsT, rhs) in enumerate(pairs):
                        nc.tensor.matmul(psum_c[:, 0:512], lhsT, rhs[:, 0:512],
                                         start=(k == 0), stop=(k == n - 1))
                        nc.tensor.matmul(psum_c[:, 512:768], lhsT, rhs[:, 512:768],
                                         start=(k == 0), stop=(k == n - 1))
                    # epilogue: ctx = psum + diag_num * hS ; LayerNorm ; ReLU
                    hs_in = small.tile([P, H], bf16, tag="hs_in", bufs=3, name=f"hsin{i}")
                    nc.scalar.dma_start(out=hs_in[:], in_=hs_dram[i * P:(i + 1) * P, :])
                    ctx = epi.tile([P, H], f32, tag="ctx", bufs=3, name=f"ctx{i}")
                    rs = small.tile([P, 1], f32, tag="rsum", bufs=4, name=f"rs{i}")
                    nc.vector.scalar_tensor_tensor(
                        out=ctx[:], in0=hs_in[:], scalar=diag_num[i][:], in1=psum_c[:],
                        op0=alu.mult, op1=alu.add, accum_out=rs[:])
                    nm = small.tile([P, 1], f32, tag="nmean", bufs=4, name=f"nm{i}")
                    nc.vector.tensor_scalar(out=nm[:], in0=rs[:], scalar1=-1.0 / H,
                                            scalar2=None, op0=alu.mult)
                    xm = epi.tile([P, H], f32, tag="xm", bufs=2, name=f"xm{i}")
                    nc.vector.tensor_scalar(out=xm[:], in0=ctx[:], scalar1=nm[:],
                                            scalar2=None, op0=alu.add)
                    sq = epi.tile([P, H], f32, tag="ctx", bufs=3, name=f"sq{i}")
                    vs = small.tile([P, 1], f32, tag="vsum", bufs=4, name=f"vs{i}")
                    nc.vector.scalar_tensor_tensor(
                        out=sq[:], in0=xm[:], scalar=1.0, in1=xm[:],
                        op0=alu.mult, op1=alu.mult, accum_out=vs[:])
                    std = small.tile([P, 1], f32, tag="std", bufs=4, name=f"std{i}")
                    nc.scalar.activation(out=std[:], in_=vs[:], func=AF.Sqrt,
                                         bias=eps_t[:], scale=1.0 / H)
                    rstd = small.tile([P, 1], f32, tag="rstd", bufs=4, name=f"rstd{i}")
                    nc.vector.reciprocal(out=rstd[:], in_=std[:])
                    outt = epi.tile([P, H], f32, tag="ctx", bufs=3, name=f"outt{i}")
                    if apply_gamma_beta:
                        y = epi.tile([P, H], f32, tag="xm", bufs=2, name=f"y{i}")
                        nc.vector.scalar_tensor_tensor(
                            out=y[:], in0=xm[:], scalar=rstd[:], in1=g_bc[:],
                            op0=alu.mult, op1=alu.mult)
                        y2 = epi.tile([P, H], f32, tag="ctx", bufs=3, name=f"y2{i}")
                        nc.vector.tensor_tensor(out=y2[:], in0=y[:], in1=beta_bc[:],
                                                op=alu.add)
                        nc.vector.tensor_scalar(out=outt[:], in0=y2[:], scalar1=0.0,
                                                scalar2=None, op0=alu.max)
                    else:
                        nc.vector.tensor_scalar(out=outt[:], in0=xm[:], scalar1=rstd[:],
                                                scalar2=0.0, op0=alu.mult, op1=alu.max)
                    nc.scalar.dma_start(out=out_dram[i * P:(i + 1) * P, :], in_=outt[:])

            e_phase(0)
            e_phase(1)
            bmm_phase(0)
            e_phase(2)
            bmm_phase(1)
            e_phase(3)
            bmm_phase(2)
            bmm_phase(3)

    nc.finalize()
    return nc


def _get_nc(apply_gamma_beta: bool):
    key = apply_gamma_beta
    if key not in _CACHE:
        _CACHE[key] = _build(apply_gamma_beta)
    return _CACHE[key]


def _prepare(hidden_state, adjacency, W_left, b_left, W_self, b_self,
             W_right, b_right, gamma, beta):
    bf = ml_dtypes.bfloat16
    hidden_state = np.ascontiguousarray(np.asarray(hidden_state, np.float32))
    adjacency = np.ascontiguousarray(np.asarray(adjacency, np.float32))
    gamma = np.asarray(gamma, np.float32)
    beta = np.asarray(beta, np.float32)
    trivial_gb = bool(np.all(gamma == 1.0) and np.all(beta == 0.0))
    nc = _get_nc(not trivial_gb)

    r = np.arange(P)
    lowmask = (r[:, None] > r[None, :]).astype(bf)
    upmask = (r[:, None] < r[None, :]).astype(bf)
    idmask = (r[:, None] == r[None, :]).astype(bf)

    base = {
        "wl": np.ascontiguousarray(np.asarray(W_left, np.float32)),
        "ws": np.ascontiguousarray(np.asarray(W_self, np.float32)),
        "wr": np.ascontiguousarray(np.asarray(W_right, np.float32)),
        "bl": np.asarray(b_left, np.float32).reshape(1, H),
        "bs": np.asarray(b_self, np.float32).reshape(1, H),
        "br": np.asarray(b_right, np.float32).reshape(1, H),
        "lowmask": lowmask, "upmask": upmask, "idmask": idmask,
    }
    if not trivial_gb:
        base["gamma"] = gamma.reshape(1, H)
        base["beta"] = beta.reshape(1, H)
    in_maps = [dict(base, h=hidden_state[i], A=adjacency[i]) for i in range(B)]
    return nc, in_maps


def kernel(hidden_state, adjacency, W_left, b_left, W_self, b_self,
           W_right, b_right, gamma, beta):
    from concourse.bass_utils import run_bass_kernel_spmd

    nc, in_maps = _prepare(hidden_state, adjacency, W_left, b_left, W_self, b_self,
                           W_right, b_right, gamma, beta)
    res = run_bass_kernel_spmd(nc, in_maps, core_ids=list(range(B)))
    return np.stack([res.results[i]["out"] for i in range(B)]).astype(np.float32)
